# revision 7
# baseline (speedup 1.0000x reference)
"""Trainium2 Bass kernel for nn_CA_84997402788103 (neural CA forward step).

Math (per batch, all spatial ops wrap/circular):
  y[4c+f] = conv3x3(x[c], F_f)  for fixed filters F = [ident, sobel_x, sobel_y, lap]
  h = relu(W1 @ y + b1)         (48 -> 96, 1x1)
  y2 = W2 @ h                   (96 -> 12, 1x1)
  out = x + y2 * floor(u + 0.5)

Decomposition used here (per image row r, col w):
  A = x[r-1] + 2 x[r] + x[r+1]          (vertical smooth)
  B = x[r+1] - x[r-1]                   (vertical diff)
  y_sobelx = A[w+1] - A[w-1]
  y_sobely = B[w-1] + 2 B[w] + B[w+1]
  y_lap    = (A[w-1] + 2 A[w] + A[w+1]) - 16 x
  => h-preact(w) = sum_{dx in {-1,0,1}} L1[dx].T @ [x; A; B](w+dx)
  with L1[dx][row, m]:
     x-rows (c):  (dx==0) * (W1[m, 4c+0] - 16 * W1[m, 4c+3])
     A-rows (c):  d(dx) * W1[m, 4c+1] + s(dx) * W1[m, 4c+3]
     B-rows (c):  s(dx) * W1[m, 4c+2]
  where d = (-1, 0, +1), s = (1, 2, 1).

Sharding: pure data-parallel, batch b -> NeuronCore b (8 cores).

Per-core pipeline (H=512, W=512, C=12), bf16 data path, fp32 accumulation:
  - image processed in chunks of 32 rows; a chunk is 8 j-groups of 4 rows
  - XS [96 = (8j x 12c), 6 x 514] bf16: x rows with +-1 halo rows, wrap cols
    (cast fp32->bf16 in the DMA via gpsimd SWDGE)
  - AS/BS [96, 4 x 514] bf16: vertical stencil on VectorE (1 TT + 1 STT + 1 TT)
  - CT [36 = x|A|B, 32 x 514] bf16: assembled via SBUF->SBUF DMA (per j-group)
  - mm1: per image row, 3 accumulating bf16 matmuls (dx-shifted rhs, K=36, M=96)
    -> PSUM h [96, 1024] (row pairs)
  - relu+bias evacuation split across ScalarE (ACT) and VectorE -> HS bf16
  - mm2: col-tiled 4x (tile_position (0,32g)), W2 padded to M=32 -> ps_j [128, W]
    "junky" layout: quad-row g lives at partitions 32g..32g+11, zeros elsewhere
  - ps_x [128, W]: x replicated into the same junky layout via identity matmuls
  - mask: u >= 0.5 on rows tile, replicated to 32-wide groups via broadcast DMA
  - final: OJ = ps_j * MJ + ps_x on [128, W], 4 strided DMAs out per quad
"""

import sys

sys.path.insert(0, "/opt/trn_rl_repo")

import numpy as np

import concourse.bass as bass  # noqa: E402,F401
import concourse.bacc as bacc  # noqa: E402
import concourse.mybir as mybir  # noqa: E402
from concourse.tile import TileContext  # noqa: E402

F32 = mybir.dt.float32
BF16 = mybir.dt.bfloat16
ALU = mybir.AluOpType

C = 12
HID = 96
NCORES = 8

CHUNK = 32        # image rows per chunk
JR = 4            # rows per j-group
NJ = CHUNK // JR  # 8 j-groups


def _build_body(nc, tc, x_d, u_d, l1_d, w2_d, w1b_d, e12_d, out_d, H, W):
    PW = W + 2
    n_chunks = H // CHUNK
    quads_per_chunk = CHUNK // 4
    ACT_SPLIT = (2 * W * 53) // 100  # evac columns handled by ScalarE (of 2W)

    MJ_NB = 4  # rotation slices of the persistent mask buffer

    wpool = tc.tile_pool(name="weights", bufs=1)
    spool = tc.tile_pool(name="stencil", bufs=2)
    cpool = tc.tile_pool(name="combined", bufs=2)
    hpool = tc.tile_pool(name="hs", bufs=4)
    jpool = tc.tile_pool(name="junk", bufs=4)
    psh_pool = tc.tile_pool(name="psum_h", bufs=3, space="PSUM")
    psj_pool = tc.tile_pool(name="psum_j", bufs=1, space="PSUM")
    psx_pool = tc.tile_pool(name="psum_x", bufs=1, space="PSUM")

    with wpool as wp, spool as sp, cpool as cp, hpool as hp, jpool as jp, \
            psh_pool as psp, psj_pool as psjp, psx_pool as psxp:
        # persistent mask buffer: u rows replicated x12 at junky partitions;
        # gap partitions zeroed once, then never rewritten
        mj_t = wp.tile([128, MJ_NB * W], F32)
        nc.gpsimd.memset(mj_t[:, :], 0.0)
        # --- weights (loaded once) ---
        l1_t = wp.tile([36, 3 * HID], BF16)           # [36, (dx, m)]
        nc.sync.dma_start(
            l1_t[:, :].rearrange("k (d m) -> k d m", d=3),
            l1_d[:, :, :].rearrange("d k m -> k d m"),
        )
        w2_t = wp.tile([HID, 32], BF16)
        nc.sync.dma_start(w2_t[:, :], w2_d[:, :])
        w1b_t = wp.tile([HID, 1], F32)
        nc.sync.dma_start(w1b_t[:, :], w1b_d[:, :])
        e12_t = wp.tile([C, 32], BF16)                # identity pad, for ps_x
        nc.sync.dma_start(e12_t[:, :], e12_d[:, :])

        for k in range(n_chunks):
            r_base = k * CHUNK
            # ---- load x chunk with halo into XS (fp32 -> bf16 cast DMA) ----
            XS = sp.tile([96, 6 * PW], BF16, tag="XS")
            XSv = XS[:, :].rearrange("p (r w) -> p r w", w=PW)
            for j in range(NJ):
                r0 = r_base + JR * j - 1  # first halo row
                dst = XSv[12 * j : 12 * (j + 1), :, 1 : W + 1]
                if r0 >= 0 and r0 + 6 <= H:
                    nc.gpsimd.dma_start(dst, x_d[:, r0 : r0 + 6, :])
                else:
                    for rr in range(6):  # wrap rows at image top/bottom
                        rsrc = (r0 + rr) % H
                        nc.gpsimd.dma_start(
                            XSv[12 * j : 12 * (j + 1), rr : rr + 1, 1 : W + 1],
                            x_d[:, rsrc : rsrc + 1, :],
                        )
            # wrap columns: col 0 <- col W, col W+1 <- col 1
            nc.vector.tensor_copy(XSv[:, :, 0:1], XSv[:, :, W : W + 1])
            nc.vector.tensor_copy(XSv[:, :, W + 1 : W + 2], XSv[:, :, 1:2])

            # ---- vertical stencil on DVE ----
            AS = sp.tile([96, JR * PW], BF16, tag="AS")
            BS = sp.tile([96, JR * PW], BF16, tag="BS")
            T1 = sp.tile([96, JR * PW], BF16, tag="T1")
            x_m1 = XSv[:, 0:JR, :]            # rows r-1
            x_0 = XSv[:, 1 : JR + 1, :]       # rows r
            x_p1 = XSv[:, 2 : JR + 2, :]      # rows r+1
            ASv = AS[:, :].rearrange("p (r w) -> p r w", w=PW)
            BSv = BS[:, :].rearrange("p (r w) -> p r w", w=PW)
            T1v = T1[:, :].rearrange("p (r w) -> p r w", w=PW)
            nc.vector.tensor_tensor(T1v, x_m1, x_p1, ALU.add)
            nc.vector.scalar_tensor_tensor(ASv, x_0, 2.0, T1v, ALU.mult, ALU.add)
            nc.vector.tensor_tensor(BSv, x_p1, x_m1, ALU.subtract)

            # ---- assemble combined tile CT [x|A|B] via SBUF->SBUF DMA ----
            CT = cp.tile([36, CHUNK * PW], BF16, tag="CT")
            CTv = CT[:, :].rearrange("p (r w) -> p r w", w=PW)
            for j in range(NJ):
                rows = slice(JR * j, JR * (j + 1))
                nc.sync.dma_start(
                    CTv[0:12, rows, :], XSv[12 * j : 12 * (j + 1), 1 : JR + 1, :]
                )
                nc.sync.dma_start(CTv[12:24, rows, :], ASv[12 * j : 12 * (j + 1), :, :])
                nc.sync.dma_start(CTv[24:36, rows, :], BSv[12 * j : 12 * (j + 1), :, :])

            # ---- per-quad compute ----
            for q in range(quads_per_chunk):
                # mm1 into two row-pair psum tiles, evacuate to HS
                HS = hp.tile([HID, 4 * W], BF16, tag="HS")
                for pair in range(2):
                    ps_h = psp.tile([HID, 2 * W], F32, tag="ps_h")
                    for r in range(2):
                        row = 4 * q + 2 * pair + r  # row within chunk
                        base = row * PW + 1
                        for idx, dx in enumerate((-1, 0, 1)):
                            nc.tensor.matmul(
                                ps_h[:, r * W : (r + 1) * W],
                                l1_t[:, idx * HID : (idx + 1) * HID],
                                CT[:, base + dx : base + dx + W],
                                start=(idx == 0),
                                stop=(idx == 2),
                            )
                    # relu + bias evacuation, split ACT / DVE along free dim
                    off = pair * 2 * W
                    nc.scalar.activation(
                        HS[:, off : off + ACT_SPLIT], ps_h[:, :ACT_SPLIT],
                        mybir.ActivationFunctionType.Relu,
                        bias=w1b_t[:, 0:1], scale=1.0,
                    )
                    nc.vector.tensor_scalar(
                        HS[:, off + ACT_SPLIT : off + 2 * W],
                        ps_h[:, ACT_SPLIT : 2 * W],
                        w1b_t[:, 0:1], 0.0, ALU.add, ALU.max,
                    )

                # mm2: col-tiled 4x into junky psum [128, W]; ps_x likewise
                ps_j = psjp.tile([128, W], F32, tag="ps_j")
                ps_x = psxp.tile([128, W], F32, tag="ps_x")
                r0 = r_base + 4 * q
                for g in range(4):
                    nc.tensor.matmul(
                        ps_j[32 * g : 32 * (g + 1), :],
                        w2_t[:, :],
                        HS[:, g * W : (g + 1) * W],
                        start=True, stop=True,
                        tile_position=(0, 32 * g),
                    )
                    crow = (4 * q + g) * PW + 1
                    nc.tensor.matmul(
                        ps_x[32 * g : 32 * (g + 1), :],
                        e12_t[:, :],
                        CT[0:12, crow : crow + W],
                        start=True, stop=True,
                        tile_position=(0, 32 * g),
                    )

                # mask replication: u row -> 12 junky partitions (plain-slice
                # dest, broadcast on the DRAM side), into the rotating slice
                qi = (k * quads_per_chunk + q) % MJ_NB
                mj = mj_t[:, qi * W : (qi + 1) * W]
                for g in range(4):
                    nc.sync.dma_start(
                        mj[32 * g : 32 * g + 12, :],
                        u_d[r0 + g : r0 + g + 1, :].broadcast_to([12, W]),
                    )

                # final combine: OJ = (mj >= 0.5) * ps_j + ps_x
                FT = jp.tile([128, W], BF16, tag="FT")
                nc.vector.scalar_tensor_tensor(
                    FT[:, :], mj, 0.5, ps_j[:, :], ALU.is_ge, ALU.mult
                )
                OJ = jp.tile([128, W], F32, tag="OJ")
                nc.vector.scalar_tensor_tensor(
                    OJ[:, :], FT[:, :], 1.0, ps_x[:, :], ALU.mult, ALU.add
                )

                for g in range(4):
                    nc.sync.dma_start(
                        out_d[:, r0 + g, :], OJ[32 * g : 32 * g + 12, :]
                    )


def _host_weights(w1_w, w1_b, w2_w):
    """Precompute L1[dx] [3, 36, 96] (rows = x|A|B), W2 pad [96, 32], bias, E12."""
    import ml_dtypes
    w1 = np.asarray(w1_w, np.float32)  # [96, 48], cols 4c+f
    d = np.array([-1.0, 0.0, 1.0], np.float32)
    s = np.array([1.0, 2.0, 1.0], np.float32)
    l1 = np.zeros((3, 36, HID), np.float32)
    for i in range(3):
        for c in range(C):
            if i == 1:
                l1[i, c, :] = w1[:, 4 * c + 0] - 16.0 * w1[:, 4 * c + 3]
            l1[i, 12 + c, :] = d[i] * w1[:, 4 * c + 1] + s[i] * w1[:, 4 * c + 3]
            l1[i, 24 + c, :] = s[i] * w1[:, 4 * c + 2]
    w2p = np.zeros((HID, 32), np.float32)
    w2p[:, :C] = np.asarray(w2_w, np.float32).T
    e12 = np.zeros((C, 32), np.float32)
    e12[np.arange(C), np.arange(C)] = 1.0
    return (
        l1.astype(ml_dtypes.bfloat16),
        w2p.astype(ml_dtypes.bfloat16),
        np.asarray(w1_b, np.float32).reshape(HID, 1),
        e12.astype(ml_dtypes.bfloat16),
    )


_NC_CACHE = {}


def _get_nc(H, W):
    key = (H, W)
    if key in _NC_CACHE:
        return _NC_CACHE[key]
    nc = bacc.Bacc("TRN2", target_bir_lowering=False, debug=False)
    x_d = nc.dram_tensor("x", [C, H, W], F32, kind="ExternalInput")
    u_d = nc.dram_tensor("u", [H, W], F32, kind="ExternalInput")
    l1_d = nc.dram_tensor("l1", [3, 36, HID], BF16, kind="ExternalInput")
    w2_d = nc.dram_tensor("w2", [HID, 32], BF16, kind="ExternalInput")
    w1b_d = nc.dram_tensor("w1b", [HID, 1], F32, kind="ExternalInput")
    e12_d = nc.dram_tensor("e12", [C, 32], BF16, kind="ExternalInput")
    out_d = nc.dram_tensor("out", [C, H, W], F32, kind="ExternalOutput")
    with TileContext(nc) as tc:
        _build_body(nc, tc, x_d, u_d, l1_d, w2_d, w1b_d, e12_d, out_d, H, W)
    nc.compile()
    nc.finalize()
    _NC_CACHE[key] = nc
    return nc


def make_in_maps(x, rand_u, w1_w, w1_b, w2_w):
    l1, w2p, w1b, e12 = _host_weights(w1_w, w1_b, w2_w)
    B = x.shape[0]
    in_maps = []
    for b in range(B):
        in_maps.append({
            "x": np.ascontiguousarray(np.asarray(x[b], np.float32)),
            "u": np.ascontiguousarray(np.asarray(rand_u[b, 0], np.float32)),
            "l1": l1,
            "w2": w2p,
            "w1b": w1b,
            "e12": e12,
        })
    return in_maps


def kernel(x, rand_u, w1_w, w1_b, w2_w):
    x = np.asarray(x)
    B, c, H, W = x.shape
    assert c == C and B == NCORES and H % CHUNK == 0
    nc = _get_nc(H, W)
    in_maps = make_in_maps(x, rand_u, w1_w, w1_b, w2_w)
    from concourse import bass_utils
    res = bass_utils.run_bass_kernel_spmd(nc, in_maps, core_ids=list(range(NCORES)))
    out = np.stack([res.results[b]["out"] for b in range(B)], axis=0)
    return out.astype(np.float32)


# revision 12
# speedup vs baseline: 3642.4562x; 3642.4562x over previous
"""Trainium2 Bass kernel for nn_CA_84997402788103 (neural CA forward step).

Math (per batch, all spatial ops wrap/circular):
  y[4c+f] = conv3x3(x[c], F_f)  for fixed filters F = [ident, sobel_x, sobel_y, lap]
  h = relu(W1 @ y + b1)         (48 -> 96, 1x1)
  y2 = W2 @ h                   (96 -> 12, 1x1)
  out = x + y2 * floor(u + 0.5)

Decomposition used here (per image row r, col w):
  A = x[r-1] + 2 x[r] + x[r+1]          (vertical smooth)
  B = x[r+1] - x[r-1]                   (vertical diff)
  y_sobelx = A[w+1] - A[w-1]
  y_sobely = B[w-1] + 2 B[w] + B[w+1]
  y_lap    = (A[w-1] + 2 A[w] + A[w+1]) - 16 x
  => h-preact(w) = sum_{dx in {-1,0,1}} L1[dx].T @ [x; A; B](w+dx)
  with L1[dx][row, m]:
     x-rows (c):  (dx==0) * (W1[m, 4c+0] - 16 * W1[m, 4c+3])
     A-rows (c):  d(dx) * W1[m, 4c+1] + s(dx) * W1[m, 4c+3]
     B-rows (c):  s(dx) * W1[m, 4c+2]
  where d = (-1, 0, +1), s = (1, 2, 1).

Sharding: pure data-parallel, batch b -> NeuronCore b (8 cores).

Per-core pipeline (H=512, W=512, C=12), bf16 data path, fp32 accumulation:
  - image processed in chunks of 32 rows; a chunk is 8 j-groups of 4 rows
  - XS [96 = (8j x 12c), 6 x 514] bf16: x rows with +-1 halo rows, wrap cols
    (cast fp32->bf16 in the DMA via gpsimd SWDGE)
  - AS/BS [96, 4 x 514] bf16: vertical stencil on VectorE (1 TT + 1 STT + 1 TT)
  - CT [36 = x|A|B, 32 x 514] bf16: assembled via SBUF->SBUF DMA (per j-group)
  - mm1: per image row, 3 accumulating bf16 matmuls (dx-shifted rhs, K=36, M=96)
    -> PSUM h [96, 1024] (row pairs)
  - relu+bias evacuation split across ScalarE (ACT) and VectorE -> HS bf16
  - mm2: col-tiled 4x (tile_position (0,32g)), W2 padded to M=32 -> ps_j [128, W]
    "junky" layout: quad-row g lives at partitions 32g..32g+11, zeros elsewhere
  - ps_x [128, W]: x replicated into the same junky layout via identity matmuls
  - mask: u >= 0.5 on rows tile, replicated to 32-wide groups via broadcast DMA
  - final: OJ = ps_j * MJ + ps_x on [128, W], 4 strided DMAs out per quad
"""

import sys

sys.path.insert(0, "/opt/trn_rl_repo")

import numpy as np

import concourse.bass as bass  # noqa: E402,F401
import concourse.bacc as bacc  # noqa: E402
import concourse.mybir as mybir  # noqa: E402
from concourse.tile import TileContext  # noqa: E402

F32 = mybir.dt.float32
BF16 = mybir.dt.bfloat16
ALU = mybir.AluOpType

C = 12
HID = 96
NCORES = 8

CHUNK = 32        # image rows per chunk
JR = 4            # rows per j-group
NJ = CHUNK // JR  # 8 j-groups


def _build_body(nc, tc, x_d, u_d, l1_d, w2_d, w1b_d, e12_d, out_d, H, W, reps=1):
    PW = W + 2
    n_chunks = H // CHUNK
    quads_per_chunk = CHUNK // 4
    ACT_SPLIT = (2 * W * 53) // 100  # evac columns handled by ScalarE (of 2W)

    MJ_NB = 4  # rotation slices of the persistent mask buffer

    wpool = tc.tile_pool(name="weights", bufs=1)
    spool = tc.tile_pool(name="stencil", bufs=2)
    cpool = tc.tile_pool(name="combined", bufs=2)
    hpool = tc.tile_pool(name="hs", bufs=4)
    jpool = tc.tile_pool(name="junk", bufs=4)
    psh_pool = tc.tile_pool(name="psum_h", bufs=3, space="PSUM")
    psj_pool = tc.tile_pool(name="psum_j", bufs=1, space="PSUM")
    psx_pool = tc.tile_pool(name="psum_x", bufs=1, space="PSUM")

    with wpool as wp, spool as sp, cpool as cp, hpool as hp, jpool as jp, \
            psh_pool as psp, psj_pool as psjp, psx_pool as psxp:
        # persistent mask buffer: u rows replicated x12 at junky partitions;
        # gap partitions zeroed once, then never rewritten
        mj_t = wp.tile([128, MJ_NB * W], F32)
        nc.gpsimd.memset(mj_t[:, :], 0.0)
        # --- weights (loaded once) ---
        l1_t = wp.tile([36, 3 * HID], BF16)           # [36, (dx, m)]
        nc.sync.dma_start(
            l1_t[:, :].rearrange("k (d m) -> k d m", d=3),
            l1_d[:, :, :].rearrange("d k m -> k d m"),
        )
        w2_t = wp.tile([HID, 32], BF16)
        nc.sync.dma_start(w2_t[:, :], w2_d[:, :])
        w1b_t = wp.tile([HID, 1], F32)
        nc.sync.dma_start(w1b_t[:, :], w1b_d[:, :])
        e12_t = wp.tile([C, 32], BF16)                # identity pad, for ps_x
        nc.sync.dma_start(e12_t[:, :], e12_d[:, :])

        def emit_grid():
            for k in range(n_chunks):
                emit_chunk(k)

        def emit_chunk(k):
            r_base = k * CHUNK
            # ---- load x chunk with halo into XS (fp32 -> bf16 cast DMA) ----
            XS = sp.tile([96, 6 * PW], BF16, tag="XS")
            XSv = XS[:, :].rearrange("p (r w) -> p r w", w=PW)
            for j in range(NJ):
                r0 = r_base + JR * j - 1  # first halo row
                dst = XSv[12 * j : 12 * (j + 1), :, 1 : W + 1]
                if r0 >= 0 and r0 + 6 <= H:
                    nc.gpsimd.dma_start(dst, x_d[:, r0 : r0 + 6, :])
                else:
                    for rr in range(6):  # wrap rows at image top/bottom
                        rsrc = (r0 + rr) % H
                        nc.gpsimd.dma_start(
                            XSv[12 * j : 12 * (j + 1), rr : rr + 1, 1 : W + 1],
                            x_d[:, rsrc : rsrc + 1, :],
                        )
            # wrap columns: col 0 <- col W, col W+1 <- col 1
            nc.vector.tensor_copy(XSv[:, :, 0:1], XSv[:, :, W : W + 1])
            nc.vector.tensor_copy(XSv[:, :, W + 1 : W + 2], XSv[:, :, 1:2])

            # ---- vertical stencil on DVE ----
            AS = sp.tile([96, JR * PW], BF16, tag="AS")
            BS = sp.tile([96, JR * PW], BF16, tag="BS")
            T1 = sp.tile([96, JR * PW], BF16, tag="T1")
            x_m1 = XSv[:, 0:JR, :]            # rows r-1
            x_0 = XSv[:, 1 : JR + 1, :]       # rows r
            x_p1 = XSv[:, 2 : JR + 2, :]      # rows r+1
            ASv = AS[:, :].rearrange("p (r w) -> p r w", w=PW)
            BSv = BS[:, :].rearrange("p (r w) -> p r w", w=PW)
            T1v = T1[:, :].rearrange("p (r w) -> p r w", w=PW)
            nc.vector.tensor_tensor(T1v, x_m1, x_p1, ALU.add)
            nc.vector.scalar_tensor_tensor(ASv, x_0, 2.0, T1v, ALU.mult, ALU.add)
            nc.vector.tensor_tensor(BSv, x_p1, x_m1, ALU.subtract)

            # ---- assemble combined tile CT [x|A|B] via SBUF->SBUF DMA ----
            CT = cp.tile([36, CHUNK * PW], BF16, tag="CT")
            CTv = CT[:, :].rearrange("p (r w) -> p r w", w=PW)
            for j in range(NJ):
                rows = slice(JR * j, JR * (j + 1))
                nc.sync.dma_start(
                    CTv[0:12, rows, :], XSv[12 * j : 12 * (j + 1), 1 : JR + 1, :]
                )
                nc.sync.dma_start(CTv[12:24, rows, :], ASv[12 * j : 12 * (j + 1), :, :])
                nc.sync.dma_start(CTv[24:36, rows, :], BSv[12 * j : 12 * (j + 1), :, :])

            # ---- per-quad compute ----
            for q in range(quads_per_chunk):
                # mm1 into two row-pair psum tiles, evacuate to HS
                HS = hp.tile([HID, 4 * W], BF16, tag="HS")
                for pair in range(2):
                    ps_h = psp.tile([HID, 2 * W], F32, tag="ps_h")
                    for r in range(2):
                        row = 4 * q + 2 * pair + r  # row within chunk
                        base = row * PW + 1
                        for idx, dx in enumerate((-1, 0, 1)):
                            nc.tensor.matmul(
                                ps_h[:, r * W : (r + 1) * W],
                                l1_t[:, idx * HID : (idx + 1) * HID],
                                CT[:, base + dx : base + dx + W],
                                start=(idx == 0),
                                stop=(idx == 2),
                            )
                    # relu + bias evacuation, split ACT / DVE along free dim
                    off = pair * 2 * W
                    nc.scalar.activation(
                        HS[:, off : off + ACT_SPLIT], ps_h[:, :ACT_SPLIT],
                        mybir.ActivationFunctionType.Relu,
                        bias=w1b_t[:, 0:1], scale=1.0,
                    )
                    nc.vector.tensor_scalar(
                        HS[:, off + ACT_SPLIT : off + 2 * W],
                        ps_h[:, ACT_SPLIT : 2 * W],
                        w1b_t[:, 0:1], 0.0, ALU.add, ALU.max,
                    )

                # mm2: col-tiled 4x into junky psum [128, W]; ps_x likewise
                ps_j = psjp.tile([128, W], F32, tag="ps_j")
                ps_x = psxp.tile([128, W], F32, tag="ps_x")
                r0 = r_base + 4 * q
                for g in range(4):
                    nc.tensor.matmul(
                        ps_j[32 * g : 32 * (g + 1), :],
                        w2_t[:, :],
                        HS[:, g * W : (g + 1) * W],
                        start=True, stop=True,
                        tile_position=(0, 32 * g),
                    )
                    crow = (4 * q + g) * PW + 1
                    nc.tensor.matmul(
                        ps_x[32 * g : 32 * (g + 1), :],
                        e12_t[:, :],
                        CT[0:12, crow : crow + W],
                        start=True, stop=True,
                        tile_position=(0, 32 * g),
                    )

                # mask replication: u row -> 12 junky partitions (plain-slice
                # dest, broadcast on the DRAM side), into the rotating slice
                qi = (k * quads_per_chunk + q) % MJ_NB
                mj = mj_t[:, qi * W : (qi + 1) * W]
                for g in range(4):
                    nc.sync.dma_start(
                        mj[32 * g : 32 * g + 12, :],
                        u_d[r0 + g : r0 + g + 1, :].broadcast_to([12, W]),
                    )

                # final combine: OJ = (mj >= 0.5) * ps_j + ps_x
                FT = jp.tile([128, W], BF16, tag="FT")
                nc.vector.scalar_tensor_tensor(
                    FT[:, :], mj, 0.5, ps_j[:, :], ALU.is_ge, ALU.mult
                )
                OJ = jp.tile([128, W], F32, tag="OJ")
                nc.vector.scalar_tensor_tensor(
                    OJ[:, :], FT[:, :], 1.0, ps_x[:, :], ALU.mult, ALU.add
                )

                for g in range(4):
                    nc.sync.dma_start(
                        out_d[:, r0 + g, :], OJ[32 * g : 32 * g + 12, :]
                    )

        if reps == 1:
            emit_grid()
        else:
            with tc.For_i(0, reps, 1):
                emit_grid()


def _host_weights(w1_w, w1_b, w2_w):
    """Precompute L1[dx] [3, 36, 96] (rows = x|A|B), W2 pad [96, 32], bias, E12."""
    import ml_dtypes
    w1 = np.asarray(w1_w, np.float32)  # [96, 48], cols 4c+f
    d = np.array([-1.0, 0.0, 1.0], np.float32)
    s = np.array([1.0, 2.0, 1.0], np.float32)
    l1 = np.zeros((3, 36, HID), np.float32)
    for i in range(3):
        for c in range(C):
            if i == 1:
                l1[i, c, :] = w1[:, 4 * c + 0] - 16.0 * w1[:, 4 * c + 3]
            l1[i, 12 + c, :] = d[i] * w1[:, 4 * c + 1] + s[i] * w1[:, 4 * c + 3]
            l1[i, 24 + c, :] = s[i] * w1[:, 4 * c + 2]
    w2p = np.zeros((HID, 32), np.float32)
    w2p[:, :C] = np.asarray(w2_w, np.float32).T
    e12 = np.zeros((C, 32), np.float32)
    e12[np.arange(C), np.arange(C)] = 1.0
    return (
        l1.astype(ml_dtypes.bfloat16),
        w2p.astype(ml_dtypes.bfloat16),
        np.asarray(w1_b, np.float32).reshape(HID, 1),
        e12.astype(ml_dtypes.bfloat16),
    )


_NC_CACHE = {}


def _get_nc(H, W, reps=1):
    key = (H, W, reps)
    if key in _NC_CACHE:
        return _NC_CACHE[key]
    nc = bacc.Bacc("TRN2", target_bir_lowering=False, debug=False)
    x_d = nc.dram_tensor("x", [C, H, W], F32, kind="ExternalInput")
    u_d = nc.dram_tensor("u", [H, W], F32, kind="ExternalInput")
    l1_d = nc.dram_tensor("l1", [3, 36, HID], BF16, kind="ExternalInput")
    w2_d = nc.dram_tensor("w2", [HID, 32], BF16, kind="ExternalInput")
    w1b_d = nc.dram_tensor("w1b", [HID, 1], F32, kind="ExternalInput")
    e12_d = nc.dram_tensor("e12", [C, 32], BF16, kind="ExternalInput")
    out_d = nc.dram_tensor("out", [C, H, W], F32, kind="ExternalOutput")
    with TileContext(nc) as tc:
        _build_body(nc, tc, x_d, u_d, l1_d, w2_d, w1b_d, e12_d, out_d, H, W,
                    reps=reps)
    nc.compile()
    nc.finalize()
    _NC_CACHE[key] = nc
    return nc


def make_in_maps(x, rand_u, w1_w, w1_b, w2_w):
    l1, w2p, w1b, e12 = _host_weights(w1_w, w1_b, w2_w)
    B = x.shape[0]
    in_maps = []
    for b in range(B):
        in_maps.append({
            "x": np.ascontiguousarray(np.asarray(x[b], np.float32)),
            "u": np.ascontiguousarray(np.asarray(rand_u[b, 0], np.float32)),
            "l1": l1,
            "w2": w2p,
            "w1b": w1b,
            "e12": e12,
        })
    return in_maps


def kernel(x, rand_u, w1_w, w1_b, w2_w):
    x = np.asarray(x)
    B, c, H, W = x.shape
    assert c == C and B == NCORES and H % CHUNK == 0
    nc = _get_nc(H, W)
    in_maps = make_in_maps(x, rand_u, w1_w, w1_b, w2_w)
    from concourse import bass_utils
    res = bass_utils.run_bass_kernel_spmd(nc, in_maps, core_ids=list(range(NCORES)))
    out = np.stack([res.results[b]["out"] for b in range(B)], axis=0)
    return out.astype(np.float32)
